# revision 2
# baseline (speedup 1.0000x reference)
"""Autoregressive GRU decoder kernel for Trainium2 (8 NeuronCores, data-parallel).

Reference semantics (B=256, T=512, X=64, H=256):
    h = h_enc; x = 0
    for t in range(T):
        gi = x @ W_ih.T + b_ih          # [B, 3H]
        gh = h @ W_hh.T + b_hh
        r = sigmoid(gi_r + gh_r); z = sigmoid(gi_z + gh_z)
        n = tanh(gi_n + r * gh_n)
        h = (1-z)*n + z*h = h + (1-z)*(n-h)
        y[t] = h @ W_out.T + b_out; x = y[t]
    out[b, t, x] = y[t][b, x]

`input` is only used for its shape in the reference; its values are unused.

Sharding: batch 256 -> 8 cores x 32. Weights replicated. The sequential scan
runs fully on-chip per core in a "transposed" layout: state h is kept as
hT[p, c*32+b] = h[b, 128*c+p] (H=256 split into 2 chunks of 128 partitions),
and the feedback x as xT[k, b] with a constant ones-row appended for fused
bias matmuls. Gate pre-activations are computed into PSUM [128, (c,b)] tiles
with the weight chunks as the stationary matmul operand.
"""

import numpy as np

X_DIM, H_DIM, T_STEPS, B = 64, 256, 512, 256
N_CORES = 8
BS = B // N_CORES  # 32 batch rows per core

# dtype for matmul operands (weights + activations); fp32 is exact but pays
# a slower stationary-weight load; float16 enables fast weight load (FWL).
import concourse.mybir as mybir

MM_DT = mybir.dt.float32
NP_MM_DT = np.float32

_CACHE = {}


def _build():
    """Build + compile the Bass module once. Returns (nc, meta)."""
    import concourse.tile as tile
    from concourse import bacc

    H2 = H_DIM // 2  # 128, partition chunk
    G = 3 * H_DIM  # 768

    nc = bacc.Bacc(None, target_bir_lowering=False)

    d_wih = nc.declare_dram_parameter("wih_aug", [X_DIM + 1, G], MM_DT, isOutput=False)
    d_whh = nc.declare_dram_parameter("whh_t", [H2, 2 * G], MM_DT, isOutput=False)
    d_wout = nc.declare_dram_parameter("wout_t", [H2, 2 * X_DIM], MM_DT, isOutput=False)
    d_bhhn = nc.declare_dram_parameter("bhh_n", [1, H_DIM], MM_DT, isOutput=False)
    d_boutc = nc.declare_dram_parameter("bout_col", [X_DIM, 1], mybir.dt.float32, isOutput=False)
    d_boutr = nc.declare_dram_parameter("bout_row", [1, X_DIM], MM_DT, isOutput=False)
    d_h0 = nc.declare_dram_parameter("h0_t", [H2, 2 * BS], mybir.dt.float32, isOutput=False)
    d_out = nc.declare_dram_parameter("out", [BS, T_STEPS, X_DIM], mybir.dt.float32, isOutput=True)
    out_flat = d_out.rearrange("b t x -> b (t x)")

    f32 = mybir.dt.float32

    with tile.TileContext(nc) as tc:
        with (
            tc.tile_pool(name="const", bufs=1) as const,
            tc.tile_pool(name="state", bufs=1) as state,
            tc.tile_pool(name="ew", bufs=2) as ew,
            tc.tile_pool(name="ps_g", bufs=1, space="PSUM") as ps_g,
            tc.tile_pool(name="ps_y", bufs=2, space="PSUM") as ps_y,
        ):
            # ---- constants / weights ----
            wih = const.tile([X_DIM + 1, G], MM_DT, tag="wih")
            whh = const.tile([H2, 2 * G], MM_DT, tag="whh")
            wout = const.tile([H2, 2 * X_DIM], MM_DT, tag="wout")
            bhhn = const.tile([1, H_DIM], MM_DT, tag="bhhn")
            boutc = const.tile([X_DIM, 1], f32, tag="boutc")
            boutr = const.tile([1, X_DIM], MM_DT, tag="boutr")
            ones32 = const.tile([1, BS], MM_DT, tag="ones32")
            ybuf = const.tile([BS, T_STEPS * X_DIM], f32, tag="ybuf")

            nc.sync.dma_start(out=wih, in_=d_wih[:])
            nc.sync.dma_start(out=whh, in_=d_whh[:])
            nc.sync.dma_start(out=wout, in_=d_wout[:])
            nc.sync.dma_start(out=bhhn, in_=d_bhhn[:])
            nc.sync.dma_start(out=boutc, in_=d_boutc[:])
            nc.sync.dma_start(out=boutr, in_=d_boutr[:])
            nc.vector.memset(ones32, 1.0)

            # ---- state ----
            hT = state.tile([H2, 2 * BS], f32, tag="hT")  # h[b, 128c+p] at [p, 32c+b]
            nc.sync.dma_start(out=hT, in_=d_h0[:])
            if MM_DT != f32:
                hTm = state.tile([H2, 2 * BS], MM_DT, tag="hTm")
                nc.gpsimd.tensor_copy(out=hTm, in_=hT)
            else:
                hTm = hT
            xTa = state.tile([X_DIM + 1, BS], MM_DT, tag="xTa")  # x.T with ones row
            nc.vector.memset(xTa, 0.0)
            nc.vector.memset(xTa[X_DIM : X_DIM + 1, :], 1.0)

            sig = mybir.ActivationFunctionType.Sigmoid
            tanh = mybir.ActivationFunctionType.Tanh
            ident = mybir.ActivationFunctionType.Identity

            for t in range(T_STEPS):
                p_r = ps_g.tile([H2, 2 * BS], f32, tag="p_r")
                p_z = ps_g.tile([H2, 2 * BS], f32, tag="p_z")
                p_in = ps_g.tile([H2, 2 * BS], f32, tag="p_in")
                p_hn = ps_g.tile([H2, 2 * BS], f32, tag="p_hn")

                # gate pre-activations, transposed: out[p, 32c+b]
                for c in range(2):
                    ob = slice(BS * c, BS * (c + 1))
                    # r gate: rows [0, 256)
                    g0 = H2 * c
                    nc.tensor.matmul(p_r[:, ob], wih[:, g0 : g0 + H2], xTa, start=True, stop=False)
                    nc.tensor.matmul(p_r[:, ob], whh[:, g0 : g0 + H2], hTm[:, 0:BS], start=False, stop=False)
                    nc.tensor.matmul(p_r[:, ob], whh[:, G + g0 : G + g0 + H2], hTm[:, BS : 2 * BS], start=False, stop=True)
                    # z gate: rows [256, 512)
                    g1 = H_DIM + H2 * c
                    nc.tensor.matmul(p_z[:, ob], wih[:, g1 : g1 + H2], xTa, start=True, stop=False)
                    nc.tensor.matmul(p_z[:, ob], whh[:, g1 : g1 + H2], hTm[:, 0:BS], start=False, stop=False)
                    nc.tensor.matmul(p_z[:, ob], whh[:, G + g1 : G + g1 + H2], hTm[:, BS : 2 * BS], start=False, stop=True)
                    # n gate: rows [512, 768): gi side (with b_ih_n) and gh side (+ b_hh_n)
                    g2 = 2 * H_DIM + H2 * c
                    nc.tensor.matmul(p_in[:, ob], wih[:, g2 : g2 + H2], xTa, start=True, stop=True)
                    nc.tensor.matmul(p_hn[:, ob], whh[:, g2 : g2 + H2], hTm[:, 0:BS], start=True, stop=False)
                    nc.tensor.matmul(p_hn[:, ob], whh[:, G + g2 : G + g2 + H2], hTm[:, BS : 2 * BS], start=False, stop=False)
                    nc.tensor.matmul(p_hn[:, ob], bhhn[:, H2 * c : H2 * (c + 1)], ones32, start=False, stop=True)

                # elementwise (all [128, 64] tiles)
                r_sb = ew.tile([H2, 2 * BS], f32, tag="r_sb")
                omz = ew.tile([H2, 2 * BS], f32, tag="omz")
                rn = ew.tile([H2, 2 * BS], f32, tag="rn")
                npre = ew.tile([H2, 2 * BS], f32, tag="npre")
                n_sb = ew.tile([H2, 2 * BS], f32, tag="n_sb")
                nmh = ew.tile([H2, 2 * BS], f32, tag="nmh")
                t1 = ew.tile([H2, 2 * BS], f32, tag="t1")

                nc.scalar.activation(r_sb, p_r, sig)
                nc.scalar.activation(omz, p_z, sig, scale=-1.0)  # 1 - z
                nc.vector.tensor_mul(rn, r_sb, p_hn)
                nc.vector.tensor_add(npre, rn, p_in)
                nc.scalar.activation(n_sb, npre, tanh)
                nc.vector.tensor_sub(nmh, n_sb, hT)
                nc.vector.tensor_mul(t1, omz, nmh)
                nc.vector.tensor_add(hT, hT, t1)  # h += (1-z)*(n-h)
                if MM_DT != f32:
                    nc.gpsimd.tensor_copy(out=hTm, in_=hT)

                # y (feedback, transposed): yT[x, b] = sum_k W_out[x, k] h[b, k]
                p_yT = ps_y.tile([X_DIM, BS], f32, tag="p_yT")
                nc.tensor.matmul(p_yT, wout[:, 0:X_DIM], hTm[:, 0:BS], start=True, stop=False)
                nc.tensor.matmul(p_yT, wout[:, X_DIM : 2 * X_DIM], hTm[:, BS : 2 * BS], start=False, stop=True)
                nc.scalar.activation(xTa[0:X_DIM, :], p_yT, ident, bias=boutc)

                # y (batch-major, for output): y[b, x]
                p_y = ps_y.tile([BS, X_DIM], f32, tag="p_y")
                nc.tensor.matmul(p_y, hTm[:, 0:BS], wout[:, 0:X_DIM], start=True, stop=False)
                nc.tensor.matmul(p_y, hTm[:, BS : 2 * BS], wout[:, X_DIM : 2 * X_DIM], start=False, stop=False)
                nc.tensor.matmul(p_y, ones32, boutr, start=False, stop=True)
                nc.vector.tensor_copy(ybuf[:, X_DIM * t : X_DIM * (t + 1)], p_y)

                # stream finished output chunks to HBM (64 steps = 16 KB/partition)
                if (t + 1) % 64 == 0 or t == T_STEPS - 1:
                    j0 = X_DIM * (t + 1 - ((t + 1) % 64 or 64))
                    j1 = X_DIM * (t + 1)
                    nc.sync.dma_start(out=out_flat[:, j0:j1], in_=ybuf[:, j0:j1])

    nc.compile()
    return nc


def _prep_inputs(input, h_enc, W_ih, W_hh, b_ih, b_hh, W_out, b_out):
    """Host-side layout prep. Returns per-core input maps."""
    H2 = H_DIM // 2
    G = 3 * H_DIM
    f32 = np.float32

    W_ih = np.asarray(W_ih, f32)
    W_hh = np.asarray(W_hh, f32)
    b_ih = np.asarray(b_ih, f32)
    b_hh = np.asarray(b_hh, f32)
    W_out = np.asarray(W_out, f32)
    b_out = np.asarray(b_out, f32)
    h_enc = np.asarray(h_enc, f32)

    # wih_aug[k, g] = W_ih[g, k]; row X_DIM = b_ih (+ b_hh for r,z gates)
    bias = b_ih.copy()
    bias[: 2 * H_DIM] += b_hh[: 2 * H_DIM]
    wih_aug = np.concatenate([W_ih.T, bias[None, :]], axis=0).astype(NP_MM_DT)  # [65, 768]

    # whh_t[p, k*G + g] = W_hh[g, 128k + p]
    whh_t = np.empty((H2, 2 * G), f32)
    whh_t[:, :G] = W_hh.T[:H2]
    whh_t[:, G:] = W_hh.T[H2:]
    whh_t = whh_t.astype(NP_MM_DT)

    # wout_t[p, k*X + x] = W_out[x, 128k + p]
    wout_t = np.empty((H2, 2 * X_DIM), f32)
    wout_t[:, :X_DIM] = W_out.T[:H2]
    wout_t[:, X_DIM:] = W_out.T[H2:]
    wout_t = wout_t.astype(NP_MM_DT)

    bhh_n = b_hh[2 * H_DIM :][None, :].astype(NP_MM_DT)  # [1, 256]
    bout_col = b_out[:, None].astype(f32)  # [64, 1]
    bout_row = b_out[None, :].astype(NP_MM_DT)  # [1, 64]

    in_maps = []
    for i in range(N_CORES):
        hs = h_enc[i * BS : (i + 1) * BS]  # [32, 256]
        hT = hs.T  # [256, 32]
        h0_t = np.concatenate([hT[:H2], hT[H2:]], axis=1).astype(f32)  # [128, 64]
        in_maps.append(
            {
                "wih_aug": wih_aug,
                "whh_t": whh_t,
                "wout_t": wout_t,
                "bhh_n": bhh_n,
                "bout_col": bout_col,
                "bout_row": bout_row,
                "h0_t": np.ascontiguousarray(h0_t),
            }
        )
    return in_maps


def kernel(**inputs) -> np.ndarray:
    from concourse.bass_utils import run_bass_kernel_spmd

    if "nc" not in _CACHE:
        _CACHE["nc"] = _build()
    nc = _CACHE["nc"]

    in_maps = _prep_inputs(**inputs)
    res = run_bass_kernel_spmd(nc, in_maps, core_ids=list(range(N_CORES)))
    out = np.concatenate([res.results[i]["out"] for i in range(N_CORES)], axis=0)
    return out.astype(np.float32)


def run_profiled(inputs, tmpdir):
    """Like kernel() but with NTFF tracing; returns (out, BassKernelResults)."""
    from concourse.bass_utils import run_bass_kernel_spmd

    try:
        from antenv.axon_hooks import get_axon_ntff_profile_hook, set_axon_ntff_profile_hook
        from trn_agent_boot.trn_boot import _ntff_profile_via_ctypes

        if get_axon_ntff_profile_hook() is None:
            set_axon_ntff_profile_hook(_ntff_profile_via_ctypes("/opt/axon/libaxon_pjrt.so"))
    except Exception as e:  # profiling unavailable; still run
        print("profiling hook unavailable:", e)

    if "nc" not in _CACHE:
        _CACHE["nc"] = _build()
    nc = _CACHE["nc"]
    in_maps = _prep_inputs(**inputs)
    res = run_bass_kernel_spmd(nc, in_maps, core_ids=list(range(N_CORES)), trace=True, tmpdir=tmpdir)
    out = np.concatenate([res.results[i]["out"] for i in range(N_CORES)], axis=0)
    return out.astype(np.float32), res


# revision 3
# speedup vs baseline: 3.6696x; 3.6696x over previous
"""Autoregressive GRU decoder kernel for Trainium2 (8 NeuronCores, data-parallel).

Reference semantics (B=256, T=512, X=64, H=256):
    h = h_enc; x = 0
    for t in range(T):
        gi = x @ W_ih.T + b_ih          # [B, 3H]
        gh = h @ W_hh.T + b_hh
        r = sigmoid(gi_r + gh_r); z = sigmoid(gi_z + gh_z)
        n = tanh(gi_n + r * gh_n)
        h = (1-z)*n + z*h
        y[t] = h @ W_out.T + b_out; x = y[t]   # y fed back as next input
    out[b, t, x] = y[t][b, x]

`input` is only used for its shape in the reference; its values are unused.

Key algebraic fusion: for t >= 1, x(t) = W_out h(t-1) + b_out, so
    gi_rz(t) + gh_rz(t) = (W_hh + W_ih @ W_out)_rz h(t-1) + const
    gi_n(t)             = (W_ih @ W_out)_n  h(t-1) + const
which removes the y->x feedback matmuls from the serial critical path; the
output projection y(t) is computed per step but nothing depends on it.

Sharding: batch 256 -> 8 cores x 32. Weights replicated. State is kept
transposed: hT[p, c*32+b] = h[b, 128*c+p] (H=256 as 2 chunks of 128
partitions). Gate pre-activations go to PSUM [128, (c,b)] with fp16 weight
chunks as the stationary matmul operand (fp32 PSUM accumulation). All
per-gate biases are applied via ScalarE activation bias / scalar_tensor_tensor
per 128-row chunk; b_out is added to the output on the host.
"""

import numpy as np

import concourse.mybir as mybir

X_DIM, H_DIM, T_STEPS, B = 64, 256, 512, 256
N_CORES = 8
BS = B // N_CORES  # 32 batch rows per core
H2 = H_DIM // 2  # 128 partition chunk

# matmul operand dtype (PSUM accumulation is always fp32)
MM_DT = mybir.dt.float16
NP_MM_DT = np.float16

_CACHE = {}


def _build():
    import concourse.tile as tile
    from concourse import bacc

    f32 = mybir.dt.float32

    nc = bacc.Bacc(None, target_bir_lowering=False)

    # fused weights, transposed into stationary-operand layout (host-prepped)
    d_wrz = nc.declare_dram_parameter("wrz", [H2, 2 * 2 * H_DIM], MM_DT, isOutput=False)
    d_wion = nc.declare_dram_parameter("wion", [H2, 2 * H_DIM], MM_DT, isOutput=False)
    d_whhn = nc.declare_dram_parameter("whhn", [H2, 2 * H_DIM], MM_DT, isOutput=False)
    d_wrz0 = nc.declare_dram_parameter("wrz0", [H2, 2 * 2 * H_DIM], MM_DT, isOutput=False)
    d_wout = nc.declare_dram_parameter("wout", [H2, 2 * X_DIM], MM_DT, isOutput=False)
    # bias columns: [128, 2] tiles, col c = bias for gate-row chunk c
    d_br = nc.declare_dram_parameter("b_r", [H2, 2], f32, isOutput=False)
    d_bzn = nc.declare_dram_parameter("b_zn", [H2, 2], f32, isOutput=False)  # negated z bias
    d_bin = nc.declare_dram_parameter("b_in", [H2, 2], f32, isOutput=False)
    d_bhn = nc.declare_dram_parameter("b_hn", [H2, 2], f32, isOutput=False)
    d_br0 = nc.declare_dram_parameter("b_r0", [H2, 2], f32, isOutput=False)
    d_bzn0 = nc.declare_dram_parameter("b_zn0", [H2, 2], f32, isOutput=False)
    d_bin0 = nc.declare_dram_parameter("b_in0", [H2, 2], f32, isOutput=False)
    d_h0 = nc.declare_dram_parameter("h0_t", [H2, 2 * BS], f32, isOutput=False)
    d_out = nc.declare_dram_parameter("out", [BS, T_STEPS, X_DIM], f32, isOutput=True)
    out_flat = d_out.rearrange("b t x -> b (t x)")

    sig = mybir.ActivationFunctionType.Sigmoid
    tanh = mybir.ActivationFunctionType.Tanh
    add_op = mybir.AluOpType.add
    mult_op = mybir.AluOpType.mult

    with tile.TileContext(nc) as tc:
        with (
            tc.tile_pool(name="const", bufs=1) as const,
            tc.tile_pool(name="state", bufs=1) as state,
            tc.tile_pool(name="ew", bufs=2) as ew,
            tc.tile_pool(name="ps_g", bufs=1, space="PSUM") as ps_g,
            tc.tile_pool(name="ps_y", bufs=2, space="PSUM") as ps_y,
        ):
            wrz = const.tile([H2, 4 * H_DIM], MM_DT, tag="wrz")
            wion = const.tile([H2, 2 * H_DIM], MM_DT, tag="wion")
            whhn = const.tile([H2, 2 * H_DIM], MM_DT, tag="whhn")
            wrz0 = const.tile([H2, 4 * H_DIM], MM_DT, tag="wrz0")
            wout = const.tile([H2, 2 * X_DIM], MM_DT, tag="wout")
            b_r = const.tile([H2, 2], f32, tag="b_r")
            b_zn = const.tile([H2, 2], f32, tag="b_zn")
            b_in = const.tile([H2, 2], f32, tag="b_in")
            b_hn = const.tile([H2, 2], f32, tag="b_hn")
            b_r0 = const.tile([H2, 2], f32, tag="b_r0")
            b_zn0 = const.tile([H2, 2], f32, tag="b_zn0")
            b_in0 = const.tile([H2, 2], f32, tag="b_in0")
            ybuf = const.tile([BS, T_STEPS * X_DIM], f32, tag="ybuf")

            for dst, src in [
                (wrz, d_wrz), (wion, d_wion), (whhn, d_whhn), (wrz0, d_wrz0),
                (wout, d_wout), (b_r, d_br), (b_zn, d_bzn), (b_in, d_bin),
                (b_hn, d_bhn), (b_r0, d_br0), (b_zn0, d_bzn0), (b_in0, d_bin0),
            ]:
                nc.sync.dma_start(out=dst, in_=src[:])

            # state: hT fp16 (matmul operand + elementwise state)
            hT = state.tile([H2, 2 * BS], MM_DT, tag="hT")
            h0f = state.tile([H2, 2 * BS], f32, tag="h0f")
            nc.sync.dma_start(out=h0f, in_=d_h0[:])
            nc.vector.tensor_copy(hT, h0f)

            for t in range(T_STEPS):
                p_r = ps_g.tile([H2, 2 * BS], f32, tag="p_r")
                p_z = ps_g.tile([H2, 2 * BS], f32, tag="p_z")
                p_in = ps_g.tile([H2, 2 * BS], f32, tag="p_in")
                p_hn = ps_g.tile([H2, 2 * BS], f32, tag="p_hn")

                w_rz_t = wrz0 if t == 0 else wrz
                # gate matmuls: out[p, 32c+b]; stationary = weight chunk [128, 128]
                for c in range(2):
                    ob = slice(BS * c, BS * (c + 1))
                    gr = H2 * c            # r rows [0, 256)
                    gz = H_DIM + H2 * c    # z rows [256, 512) -> wrz cols offset
                    gn = H2 * c            # n rows within [0, 256) of the n-blocks
                    for k in range(2):
                        kb = slice(BS * k, BS * (k + 1))
                        koff = 2 * H_DIM * k
                        nc.tensor.matmul(p_r[:, ob], w_rz_t[:, koff + gr : koff + gr + H2],
                                         hT[:, kb], start=(k == 0), stop=(k == 1))
                        nc.tensor.matmul(p_z[:, ob], w_rz_t[:, koff + gz : koff + gz + H2],
                                         hT[:, kb], start=(k == 0), stop=(k == 1))
                        nc.tensor.matmul(p_hn[:, ob], whhn[:, H_DIM * k + gn : H_DIM * k + gn + H2],
                                         hT[:, kb], start=(k == 0), stop=(k == 1))
                        if t > 0:
                            nc.tensor.matmul(p_in[:, ob], wion[:, H_DIM * k + gn : H_DIM * k + gn + H2],
                                             hT[:, kb], start=(k == 0), stop=(k == 1))

                br_t = b_r0 if t == 0 else b_r
                bzn_t = b_zn0 if t == 0 else b_zn
                bin_t = b_in0 if t == 0 else b_in

                r_sb = ew.tile([H2, 2 * BS], f32, tag="r_sb")
                omz = ew.tile([H2, 2 * BS], f32, tag="omz")
                rn = ew.tile([H2, 2 * BS], f32, tag="rn")
                npre = ew.tile([H2, 2 * BS], f32, tag="npre")
                n_sb = ew.tile([H2, 2 * BS], f32, tag="n_sb")
                nmh = ew.tile([H2, 2 * BS], f32, tag="nmh")
                t1 = ew.tile([H2, 2 * BS], f32, tag="t1")

                # two independent per-chunk chains (c = 0, 1), pipelined on S/V
                for c in range(2):
                    ob = slice(BS * c, BS * (c + 1))
                    bc = slice(c, c + 1)
                    # r = sigmoid(p_r + b_r)
                    nc.scalar.activation(r_sb[:, ob], p_r[:, ob], sig, bias=br_t[:, bc])
                    # 1-z = sigmoid(-p_z - b_z)
                    nc.scalar.activation(omz[:, ob], p_z[:, ob], sig, scale=-1.0, bias=bzn_t[:, bc])
                    # rn = (p_hn + b_hn) * r
                    nc.vector.scalar_tensor_tensor(rn[:, ob], p_hn[:, ob], b_hn[:, bc],
                                                   r_sb[:, ob], op0=add_op, op1=mult_op)
                    if t > 0:
                        nc.vector.tensor_add(npre[:, ob], rn[:, ob], p_in[:, ob])
                        nc.scalar.activation(n_sb[:, ob], npre[:, ob], tanh, bias=bin_t[:, bc])
                    else:
                        nc.scalar.activation(n_sb[:, ob], rn[:, ob], tanh, bias=bin_t[:, bc])
                    # h' = h + (1-z)*(n - h)
                    nc.gpsimd.tensor_sub(nmh[:, ob], n_sb[:, ob], hT[:, ob])
                    nc.vector.tensor_mul(t1[:, ob], omz[:, ob], nmh[:, ob])
                    nc.vector.tensor_add(hT[:, ob], hT[:, ob], t1[:, ob])

                # y(t) = W_out h(t)  (bias added on host); off the critical path
                p_y = ps_y.tile([BS, X_DIM], f32, tag="p_y")
                nc.tensor.matmul(p_y, hT[:, 0:BS], wout[:, 0:X_DIM], start=True, stop=False)
                nc.tensor.matmul(p_y, hT[:, BS : 2 * BS], wout[:, X_DIM : 2 * X_DIM], start=False, stop=True)
                nc.scalar.copy(ybuf[:, X_DIM * t : X_DIM * (t + 1)], p_y)

                if (t + 1) % 64 == 0 or t == T_STEPS - 1:
                    j0 = X_DIM * (t + 1 - ((t + 1) % 64 or 64))
                    j1 = X_DIM * (t + 1)
                    nc.sync.dma_start(out=out_flat[:, j0:j1], in_=ybuf[:, j0:j1])

    nc.compile()
    return nc


def _prep_inputs(input, h_enc, W_ih, W_hh, b_ih, b_hh, W_out, b_out):
    f64 = np.float64
    W_ih = np.asarray(W_ih, f64)
    W_hh = np.asarray(W_hh, f64)
    b_ih = np.asarray(b_ih, f64)
    b_hh = np.asarray(b_hh, f64)
    W_out64 = np.asarray(W_out, f64)
    b_out64 = np.asarray(b_out, f64)
    h_enc = np.asarray(h_enc, np.float32)

    W_io = W_ih @ W_out64          # [768, 256]
    b_io = W_ih @ b_out64          # [768]
    W_comb = W_hh + W_io           # valid for r,z rows

    def stat_layout(W):  # [rows, 256] -> [128, 2*rows] stationary layout
        Wt = W.T  # [256, rows]
        return np.concatenate([Wt[:H2], Wt[H2:]], axis=1)

    wrz = stat_layout(W_comb[: 2 * H_DIM]).astype(NP_MM_DT)        # [128, 1024]
    wion = stat_layout(W_io[2 * H_DIM :]).astype(NP_MM_DT)         # [128, 512]
    whhn = stat_layout(W_hh[2 * H_DIM :]).astype(NP_MM_DT)         # [128, 512]
    wrz0 = stat_layout(W_hh[: 2 * H_DIM]).astype(NP_MM_DT)         # [128, 1024]
    woutl = stat_layout(W_out64).astype(NP_MM_DT)                  # [128, 128]

    def cols(v):  # [256] -> [128, 2] (col c = rows 128c..128c+128)
        return np.stack([v[:H2], v[H2:]], axis=1).astype(np.float32)

    bias_rz = b_ih[: 2 * H_DIM] + b_hh[: 2 * H_DIM]
    b_r1 = cols(bias_rz[:H_DIM] + b_io[:H_DIM])
    b_zn1 = cols(-(bias_rz[H_DIM:] + b_io[H_DIM : 2 * H_DIM]))
    b_in1 = cols(b_ih[2 * H_DIM :] + b_io[2 * H_DIM :])
    b_hn = cols(b_hh[2 * H_DIM :])
    b_r0 = cols(bias_rz[:H_DIM])
    b_zn0 = cols(-bias_rz[H_DIM:])
    b_in0 = cols(b_ih[2 * H_DIM :])

    shared = {
        "wrz": wrz, "wion": wion, "whhn": whhn, "wrz0": wrz0, "wout": woutl,
        "b_r": b_r1, "b_zn": b_zn1, "b_in": b_in1, "b_hn": b_hn,
        "b_r0": b_r0, "b_zn0": b_zn0, "b_in0": b_in0,
    }
    in_maps = []
    for i in range(N_CORES):
        hT = h_enc[i * BS : (i + 1) * BS].T  # [256, 32]
        h0_t = np.ascontiguousarray(np.concatenate([hT[:H2], hT[H2:]], axis=1), dtype=np.float32)
        in_maps.append({**shared, "h0_t": h0_t})
    return in_maps, np.asarray(b_out64, np.float32)


def kernel(**inputs) -> np.ndarray:
    from concourse.bass_utils import run_bass_kernel_spmd

    if "nc" not in _CACHE:
        _CACHE["nc"] = _build()
    nc = _CACHE["nc"]
    in_maps, b_out = _prep_inputs(**inputs)
    res = run_bass_kernel_spmd(nc, in_maps, core_ids=list(range(N_CORES)))
    out = np.concatenate([res.results[i]["out"] for i in range(N_CORES)], axis=0)
    return (out + b_out[None, None, :]).astype(np.float32)


def run_profiled(inputs, tmpdir):
    from concourse.bass_utils import run_bass_kernel_spmd

    try:
        from antenv.axon_hooks import get_axon_ntff_profile_hook, set_axon_ntff_profile_hook
        from trn_agent_boot.trn_boot import _ntff_profile_via_ctypes

        if get_axon_ntff_profile_hook() is None:
            set_axon_ntff_profile_hook(_ntff_profile_via_ctypes("/opt/axon/libaxon_pjrt.so"))
    except Exception as e:
        print("profiling hook unavailable:", e)

    if "nc" not in _CACHE:
        _CACHE["nc"] = _build()
    nc = _CACHE["nc"]
    in_maps, b_out = _prep_inputs(**inputs)
    res = run_bass_kernel_spmd(nc, in_maps, core_ids=list(range(N_CORES)), trace=True, tmpdir=tmpdir)
    out = np.concatenate([res.results[i]["out"] for i in range(N_CORES)], axis=0)
    return (out + b_out[None, None, :]).astype(np.float32), res
